# revision 17
# baseline (speedup 1.0000x reference)
# Multi-head attention (B=2, S=4096, D=768, H=12) on 8 Trainium2 NeuronCores.
#
# Sharding: 24 (batch, head) units -> 3 heads x 1 batch per core.
#   core c: batch b = c // 4, heads h0..h0+2 where h0 = 3 * (c % 4).
# Each core computes q/k/v projections for its heads, attention, and a
# row-parallel partial of the output projection (its 192 columns of the
# concat dimension).  Host sums the 4 partials per batch and adds bo.
#
# Engine split for softmax exp (the co-bottleneck with the PE):
#   - lane-0 units exp on ScalarE (ACT), lane-1 units exp on VectorE via a
#     custom 8-op DVE instruction that constructs fp16 BITS of exp directly
#     (floor-capture magic-add + endpoint-constrained quadratic mantissa),
#     followed by an int16 constant-subtract; PV reads the int16 tile
#     bitcast as fp16.  A 256-column ACT helper slice balances engines.
#   - scores arrive pre-scaled by ALPHA (folded into Wk host-side); both
#     exp paths emit weights scaled by 2^-6 (cancels in softmax).
#   - softmax rowsums ride a ones-column in V (free on PE); reciprocals are
#     batched into one [rows,512] tile + reciprocal_approx_fast.
import numpy as np

D_MODEL = 768
NUM_HEADS = 12
DK = 64
B = 2
S_FULL = 4096
N_CORES = 8
HPC = 3  # heads per core
CT = D_MODEL // 128  # contraction tiles for projections

# ---- custom DVE exp-bits constants ----
M_MAGIC = 1.5 * 2**33
PRESHIFT = 17920.0  # Be(18432) - 512
C2_FIT = 0.00033312047729328157
K2_SUB = float(np.rint(8704.0 + 262144.0 * C2_FIT))
ALPHA = 1024.0 * 0.125 / np.log(2)  # raw-score -> bits scale (baked into Wk)
LN2 = float(np.log(2))
ACT_HELPER = 256
DEBUG_DUMP = False
FORCE_ACT = False  # columns of each DVE-stream chunk handled by ACT instead


def _exp_ref(in0, in1, s0, s1, imm2):
    y = (in0.astype(np.float32) + np.float32(s0)).astype(np.float32)
    t = (y + np.float32(s1)).astype(np.float32)
    r0 = (t - np.float32(s1)).astype(np.float32)
    h = (y - r0).astype(np.float32)
    q = (np.float32(imm2) * h).astype(np.float32)
    p = (q * h).astype(np.float32)
    u = (p + h).astype(np.float32)
    return (u + r0).astype(np.float32)


def make_exp_op():
    import concourse.dve_ops as dve_ops_mod
    from concourse.dve_ops import DveOp
    from concourse.dve_spec import C0, C1, C2, Spec, Src0, lower
    from concourse.dve_uop import DveOpSpec

    name = "EXP_BITS_ANT"
    if name in dve_ops_mod._SUB_OPCODE_FOR_NAME:
        return next(o for o in dve_ops_mod.OPS if o.name == name)
    y = Src0 + C0
    t = y + C1
    r0 = t - C1
    h = y - r0
    q = C2 * h
    p = q * h
    u = p + h
    body = u + r0
    spec = Spec(body=body, reference=_exp_ref)
    row = max(dve_ops_mod._SUB_OPCODE_FOR_NAME.values()) + 1
    assert row < 0x20, row
    tmp = DveOpSpec(name=name, opcode=row, uops=lower(spec, ver="v3"),
                    rd1_en=False)
    op = DveOp(name, spec, subdim=False, uops_sha={"v3": tmp.sha("v3")})
    dve_ops_mod.OPS.append(op)
    dve_ops_mod._SUB_OPCODE_FOR_NAME[name] = row
    dve_ops_mod.CUSTOM_DVE_SPECS[name] = op.spec
    return op


def _chunk_sizes(ktiles):
    # 3 k-tiles per exp chunk; two independent streams each own a 3-bank
    # psum slot + a 1-bank output accumulator (3+3+1+1 = 8 banks)
    out = []
    rem = ktiles
    if rem % 3:
        out.append(rem % 3)
        rem -= rem % 3
    while rem > 0:
        out.append(3)
        rem -= 3
    return out


def _emit(nc, tc, S):
    import concourse.mybir as mybir
    from contextlib import ExitStack

    exp_op = make_exp_op()
    f32 = mybir.dt.float32
    fr = mybir.dt.float16
    i16 = mybir.dt.int16
    u16 = mybir.dt.uint16
    Exp = mybir.ActivationFunctionType.Exp
    ADD = mybir.AluOpType.add
    SUB = mybir.AluOpType.subtract
    MAX = mybir.AluOpType.max

    QB = S // 512  # 512-query blocks
    ST = S // 128  # 128-row tiles of S (also k-tiles)
    CHUNKS = _chunk_sizes(ST)
    N_UNITS = HPC * QB

    # ---- DRAM I/O ----
    xq = nc.dram_tensor("xq_t", [D_MODEL, S], fr, kind="ExternalInput")
    xk = nc.dram_tensor("xk_t", [D_MODEL, S], fr, kind="ExternalInput")
    xv = nc.dram_tensor("xv_t", [D_MODEL, S], fr, kind="ExternalInput")
    wq = nc.dram_tensor("wq_t", [D_MODEL, 256], fr, kind="ExternalInput")
    wk = nc.dram_tensor("wk_t", [D_MODEL, 256], fr, kind="ExternalInput")
    wv = nc.dram_tensor("wv_t", [D_MODEL, 256], fr, kind="ExternalInput")
    wo = nc.dram_tensor("wo_t", [DK, HPC, D_MODEL], fr, kind="ExternalInput")
    bqd = nc.dram_tensor("bq_p", [128, 2], f32, kind="ExternalInput")
    bkd = nc.dram_tensor("bk_p", [128, 2], f32, kind="ExternalInput")
    bvd = nc.dram_tensor("bv_p", [128, HPC * DK], f32, kind="ExternalInput")
    y_out = nc.dram_tensor("y_out", [S, D_MODEL], fr, kind="ExternalOutput")

    ctx = ExitStack()
    with ctx:
        persist = ctx.enter_context(tc.tile_pool(name="persist", bufs=1))
        xpool = ctx.enter_context(tc.tile_pool(name="xpool", bufs=4))
        ptpool = ctx.enter_context(tc.tile_pool(name="ptpool", bufs=4))
        spool = ctx.enter_context(tc.tile_pool(name="spool", bufs=2))
        ps = ctx.enter_context(tc.tile_pool(name="ps", bufs=1, space="PSUM"))

        def s_slot(i):
            return ps.tile([128, 1536], f32, tag=("s3a" if i % 2 == 0 else "s3b"),
                           name=f"sslot{i % 2}")

        def o_slot(i):
            return ps.tile([128, 512], f32, tag=("oa" if i % 2 == 0 else "ob"),
                           name=f"oslot{i % 2}")

        # ---- persistent SBUF ----
        wq_sb = persist.tile([128, CT, 256], fr, tag="wq_sb")
        wk_sb = persist.tile([128, CT, 256], fr, tag="wk_sb")
        wv_sb = persist.tile([128, CT, 256], fr, tag="wv_sb")
        wo_sb = persist.tile([DK, HPC, D_MODEL], fr, tag="wo_sb")
        bq_sb = persist.tile([128, 2], f32, tag="bq_sb")
        bk_sb = persist.tile([128, 2], f32, tag="bk_sb")
        bv_sb = persist.tile([128, HPC * DK], f32, tag="bv_sb")
        ones_sb = persist.tile([128, DK], fr, tag="ones_sb")
        qt01 = persist.tile([128, S], fr, tag="qt01")
        qt2 = persist.tile([128, S], fr, tag="qt2")
        kt01 = persist.tile([128, S], fr, tag="kt01")
        kt2 = persist.tile([128, S], fr, tag="kt2")
        v_all = persist.tile([128, ST, HPC, DK + 1], fr, tag="v_all")
        ot = [
            persist.tile([DK + 1, S], fr, tag=f"ot{h}", name=f"ot{h}")
            for h in range(HPC)
        ]
        ebias = persist.tile([128, 1], f32, tag="ebias")
        rs_all = persist.tile([N_UNITS, 512], fr, tag="rs_all")
        rs32 = persist.tile([N_UNITS, 512], f32, tag="rs32")
        rr_all = persist.tile([N_UNITS, 512], f32, tag="rr_all")
        rr16 = persist.tile([N_UNITS, 512], fr, tag="rr16")

        nc.sync.dma_start(wq_sb[:], wq[:].rearrange("(o p) m -> p o m", p=128))
        nc.sync.dma_start(wk_sb[:], wk[:].rearrange("(o p) m -> p o m", p=128))
        nc.sync.dma_start(wv_sb[:], wv[:].rearrange("(o p) m -> p o m", p=128))
        nc.sync.dma_start(wo_sb[:], wo[:])
        nc.sync.dma_start(bq_sb[:], bqd[:])
        nc.sync.dma_start(bk_sb[:], bkd[:])
        nc.sync.dma_start(bv_sb[:], bvd[:])
        nc.vector.memset(ones_sb[:], 1.0)
        nc.vector.memset(ebias[:], -6.0 * LN2)
        nc.vector.memset(v_all[:, :, :, DK : DK + 1], 1.0)

        # ---- q/k projections (transposed form [heads*64, S]) ----
        def proj_qk_block(x_dram, w_sb, b_sb, dst01, dst2, qb, xtag):
            sl = slice(qb * 512, (qb + 1) * 512)
            xt = xpool.tile([128, CT, 512], fr, tag=xtag, name=f"xt_{xtag}")
            nc.sync.dma_start(
                xt[:], x_dram[:, sl].rearrange("(o p) s -> p o s", p=128)
            )
            slot = s_slot(qb)
            p1 = slot[:, 0:512]
            p2 = slot[:, 512:1024]
            for c in range(CT):
                nc.tensor.matmul(
                    p1, w_sb[:, c, 0:128], xt[:, c, :],
                    start=(c == 0), stop=(c == CT - 1),
                )
                nc.tensor.matmul(
                    p2, w_sb[:, c, 128:256], xt[:, c, :],
                    start=(c == 0), stop=(c == CT - 1),
                )
            nc.vector.tensor_scalar(dst01[:, sl], p1, b_sb[:, 0:1], None, ADD)
            nc.vector.tensor_scalar(dst2[:, sl], p2, b_sb[:, 1:2], None, ADD)

        # order: k first, then v, then q
        for qb in range(QB):
            proj_qk_block(xk, wk_sb, bk_sb, kt01, kt2, qb, "xk")

        # ---- v projection (natural layout [S, 64] per head) ----
        for g in range(ST // 4):
            gsl = slice(g * 512, (g + 1) * 512)
            xt = xpool.tile([128, CT, 512], fr, tag="xv")
            nc.sync.dma_start(
                xt[:], xv[:, gsl].rearrange("(o p) s -> p o s", p=128)
            )
            for st in range(g * 4, g * 4 + 4):
                off = (st % 4) * 128
                pv = s_slot(st)[:, 0:256]
                for c in range(CT):
                    nc.tensor.matmul(
                        pv, xt[:, c, off : off + 128], wv_sb[:, c, 0:256],
                        start=(c == 0), stop=(c == CT - 1),
                    )
                for h in range(HPC):
                    nc.vector.tensor_add(
                        v_all[:, st, h, 0:DK],
                        pv[:, h * DK : (h + 1) * DK],
                        bv_sb[:, h * DK : (h + 1) * DK],
                    )

        for qb in range(QB):
            proj_qk_block(xq, wq_sb, bq_sb, qt01, qt2, qb, "xq")

        # ---- attention: paired streams, QK packed as concurrent row-groups ----
        # pair (h0,qb)+(h1,qb): h0 on array rows 0-63, h1 on rows 64-127
        # pair (h2,qb)+(h2,qb'): uses qt2/kt2 whose rows 64-127 duplicate h2
        # lane 0 exp on ACT; lane 1 exp on DVE custom op (+ACT helper cols)
        def unit_aps(h, lane):
            rows = slice(0, DK) if lane == 0 else slice(DK, 128)
            if h < 2:
                return (qt01[rows, :], kt01[rows, :])
            return (qt2[rows, :], kt2[rows, :])

        def unit_state(h, qb, idx, lane):
            qt_ap, kt_ap = unit_aps(h, lane)
            return {
                "h": h, "sl": slice(qb * 512, (qb + 1) * 512),
                "po": o_slot(idx), "kk": 0, "qt": qt_ap, "kt": kt_ap,
                "lane": lane,
            }

        def emit_chunk_qk(p_s, st_, j):
            kk = st_["kk"]
            kt_sl = slice((kk + j) * 128, (kk + j + 1) * 128)
            nc.tensor.matmul(
                p_s[:, j * 512 : (j + 1) * 512],
                st_["kt"][:, kt_sl], st_["qt"][:, st_["sl"]],
                start=True, stop=True,
            )

        def emit_chunk_act(p_s, idx, st_, cs):
            W = cs * 512
            if st_["lane"] == 0 or FORCE_ACT:
                pt = ptpool.tile([128, 1536], fr, tag=f"ptA{idx}",
                                 name=f"ptA{idx}")
                nc.scalar.activation(pt[:, :W], p_s[:, :W], Exp,
                                     bias=ebias[:], scale=LN2 / 1024.0)
                st_["pv_pend"] = (pt, None, st_["kk"], cs)
            else:
                raw = ptpool.tile([128, 1536], u16, tag="rawB", name="rawB")
                pt = ptpool.tile([128, 1536], i16, tag=f"ptB{idx % 2}",
                                 name=f"ptB{idx % 2}")
                split = W - ACT_HELPER
                nc.vector._custom_dve(
                    exp_op, out=raw[:, 0:split], in0=p_s[:, 0:split],
                    s0=PRESHIFT, s1=M_MAGIC, imm2=C2_FIT,
                )
                nc.vector.tensor_scalar(
                    pt[:, 0:split], raw[:, 0:split], K2_SUB, 0.0, SUB, MAX
                )
                if ACT_HELPER:
                    nc.scalar.activation(
                        pt[:, split:W].bitcast(fr), p_s[:, split:W], Exp,
                        bias=ebias[:], scale=LN2 / 1024.0,
                    )
                st_["pv_pend"] = (pt, fr, st_["kk"], cs)
            st_["kk"] += cs

        def emit_pv(st_):
            if st_.get("pv_pend") is None:
                return
            pt, cast, kk, cs = st_["pv_pend"]
            h, po = st_["h"], st_["po"]
            pt_ap = pt[:].bitcast(cast) if cast is not None else pt[:]
            for j in range(cs):
                nc.tensor.matmul(
                    po[0 : DK + 1, :],
                    v_all[:, kk + j, h, :],
                    pt_ap[:, j * 512 : (j + 1) * 512],
                    start=(kk + j == 0), stop=(kk + j == ST - 1),
                )
            st_["pv_pend"] = None

        # finish: copy PV psum out, stash rowsum row; recip + normalize are
        # batched per 2 pairs (4 units)
        norm_pend = []
        rows_done = [0]  # units copied so far (== rs_all rows filled)

        def finish_copy(st_, row):
            h, sl, po = st_["h"], st_["sl"], st_["po"]
            nc.vector.tensor_copy(ot[h][0 : DK + 1, sl], po[0 : DK + 1, :])
            nc.sync.dma_start(rs_all[row : row + 1, :], ot[h][DK : DK + 1, sl])
            norm_pend.append((st_["h"], st_["sl"], row))
            rows_done[0] = row + 1

        def flush_norms():
            if not norm_pend:
                return
            # whole-tile ops: DVE cost is per-lane FD, and engine APs must be
            # 32-partition aligned; unfilled rows are garbage-in/garbage-out
            nc.vector.tensor_copy(rs32[:], rs_all[:])
            nc.vector.reciprocal_approx_fast(rr_all[:], rs32[:])
            nc.vector.tensor_copy(rr16[:], rr_all[:])
            for h, sl, row in norm_pend:
                rb1 = spool.tile([1, 512], fr, tag="rb1")
                nc.sync.dma_start(rb1[:], rr16[row : row + 1, :])
                rbc = spool.tile([DK, 512], fr, tag="rbc")
                nc.gpsimd.partition_broadcast(rbc[:], rb1[0:1, :])
                nc.vector.tensor_mul(ot[h][0:DK, sl], ot[h][0:DK, sl], rbc[:])
            norm_pend.clear()

        pairs = [((0, qb), (1, qb)) for qb in range(QB)]
        h2qbs = list(range(QB))
        while len(h2qbs) >= 2:
            pairs.append(((2, h2qbs.pop(0)), (2, h2qbs.pop(0))))
        solo = [(2, qb) for qb in h2qbs]

        def emit_y(qts):
            for qt in qts:
                q_sl = slice(qt * 128, (qt + 1) * 128)
                py = s_slot(qt)[:, 0:768]
                for h in range(HPC):
                    nc.tensor.matmul(
                        py[:, 0:512], ot[h][0:DK, q_sl], wo_sb[:, h, 0:512],
                        start=(h == 0), stop=(h == HPC - 1),
                    )
                    nc.tensor.matmul(
                        py[:, 512:768], ot[h][0:DK, q_sl], wo_sb[:, h, 512:768],
                        start=(h == 0), stop=(h == HPC - 1),
                    )
                ysb = spool.tile([128, D_MODEL], fr, tag="ysb", bufs=3)
                nc.scalar.copy(ysb[:], py)
                nc.sync.dma_start(y_out[q_sl, :], ysb[:])

        pending = None
        for pi, ((hA, qbA), (hB, qbB)) in enumerate(pairs):
            stA = unit_state(hA, qbA, 0, 0)
            stB = unit_state(hB, qbB, 1, 1)
            for ci, cs in enumerate(CHUNKS):
                psA = s_slot(0)
                psB = s_slot(1)
                for j in range(cs):
                    emit_chunk_qk(psA, stA, j)
                    emit_chunk_qk(psB, stB, j)
                emit_pv(stA)
                emit_pv(stB)
                if DEBUG_DUMP and pi == 0 and ci == 1:
                    ps_d = nc.dram_tensor("psB_dump", [128, 1536], f32,
                                          kind="ExternalOutput")
                    ps_stage = persist.tile([128, 1536], f32, tag="ps_stage")
                    nc.vector.tensor_copy(ps_stage[:], psB[:])
                    nc.sync.dma_start(ps_d[:], ps_stage[:])
                emit_chunk_act(psA, 0, stA, cs)
                emit_chunk_act(psB, 1, stB, cs)
                if DEBUG_DUMP and pi == 0 and ci == 1:
                    ptB_dbg = stB["pv_pend"][0]
                    pt_d = nc.dram_tensor("ptB_dump", [128, 1536], i16,
                                          kind="ExternalOutput")
                    nc.sync.dma_start(pt_d[:], ptB_dbg[:])
                if ci == 0 and pending is not None:
                    r = rows_done[0]
                    finish_copy(pending[0], r)
                    finish_copy(pending[1], r + 1)
                    pending = None
                    if len(norm_pend) >= 8:
                        flush_norms()
            emit_pv(stA)
            emit_pv(stB)
            pending = (stA, stB)
        if pending is not None:
            r = rows_done[0]
            finish_copy(pending[0], r)
            finish_copy(pending[1], r + 1)
            pending = None
        for h, qb in solo:
            stA = unit_state(h, qb, 0, 0)
            for ci, cs in enumerate(CHUNKS):
                psA = s_slot(0)
                for j in range(cs):
                    emit_chunk_qk(psA, stA, j)
                emit_pv(stA)
                emit_chunk_act(psA, 0, stA, cs)
            emit_pv(stA)
            finish_copy(stA, rows_done[0])
        flush_norms()

        # ---- output projection partials ----
        emit_y(range(ST))

        if DEBUG_DUMP:
            rs_d = nc.dram_tensor("rs_dump", [N_UNITS, 512], fr,
                                  kind="ExternalOutput")
            rr_d = nc.dram_tensor("rr_dump", [N_UNITS, 512], f32,
                                  kind="ExternalOutput")
            nc.sync.dma_start(rs_d[:], rs_all[:])
            nc.sync.dma_start(rr_d[:], rr_all[:])
            for h in range(HPC):
                ot_d = nc.dram_tensor(f"ot_dump{h}", [DK + 1, S], fr,
                                      kind="ExternalOutput")
                nc.sync.dma_start(ot_d[:], ot[h][:])


def build_nc(S=S_FULL):
    import concourse.bacc as bacc
    import concourse.tile as tile

    nc = bacc.Bacc("TRN2", target_bir_lowering=False, debug=False)
    with tile.TileContext(nc) as tc:
        _emit(nc, tc, S)
    nc.compile()
    return nc


def make_in_maps(query, key, value, Wq, bq, Wk, bk, Wv, bv, Wo, bo, S=S_FULL):
    """Per-core input dicts (host-side sharding / layout marshalling)."""
    query = np.asarray(query, dtype=np.float32)
    key = np.asarray(key, dtype=np.float32)
    value = np.asarray(value, dtype=np.float32)
    Wq, Wk, Wv, Wo = (np.asarray(w, dtype=np.float32) for w in (Wq, Wk, Wv, Wo))
    bq, bk, bv = (np.asarray(x, dtype=np.float32) for x in (bq, bk, bv))

    xq_b = [np.ascontiguousarray(query[b].T.astype(np.float16)) for b in range(B)]
    xk_b = [np.ascontiguousarray(key[b].T.astype(np.float16)) for b in range(B)]
    xv_b = [np.ascontiguousarray(value[b].T.astype(np.float16)) for b in range(B)]
    # K-projection carries the ALPHA score pre-scale for the exp bit trick
    WqT = Wq.T.astype(np.float16)
    WkT = (Wk.T * np.float32(ALPHA)).astype(np.float16)
    WvT = Wv.T.astype(np.float16)
    WoT = Wo.T.astype(np.float16)
    bk_s = bk * np.float32(ALPHA)

    in_maps = []
    for core in range(N_CORES):
        b = core // 4
        h0 = HPC * (core % 4)
        cs = slice(h0 * DK, (h0 + HPC) * DK)
        bq_p = np.zeros((128, 2), np.float32)
        bk_p = np.zeros((128, 2), np.float32)
        bq_l, bk_l, bv_l = bq[cs], bk_s[cs], bv[cs]
        bq_p[:, 0], bq_p[0:DK, 1], bq_p[DK:128, 1] = (
            bq_l[0:128], bq_l[128:192], bq_l[128:192])
        bk_p[:, 0], bk_p[0:DK, 1], bk_p[DK:128, 1] = (
            bk_l[0:128], bk_l[128:192], bk_l[128:192])
        in_maps.append({
            "xq_t": xq_b[b],
            "xk_t": xk_b[b],
            "xv_t": xv_b[b],
            "wq_t": np.concatenate(
                [WqT[:, cs], WqT[:, cs.start + 2 * DK : cs.stop]], axis=1
            ),
            "wk_t": np.concatenate(
                [WkT[:, cs], WkT[:, cs.start + 2 * DK : cs.stop]], axis=1
            ),
            "wv_t": np.concatenate(
                [WvT[:, cs], np.zeros((D_MODEL, 256 - HPC * DK), np.float16)], axis=1
            ),
            "wo_t": np.ascontiguousarray(
                WoT[cs, :].reshape(HPC, DK, D_MODEL).transpose(1, 0, 2)
            ),
            "bq_p": bq_p,
            "bk_p": bk_p,
            "bv_p": np.tile(bv_l[None, :], (128, 1)).astype(np.float32),
        })
    return in_maps


_NC_CACHE = {}


def kernel(query, key, value, Wq, bq, Wk, bk, Wv, bv, Wo, bo):
    from concourse import bass_utils

    if S_FULL not in _NC_CACHE:
        _NC_CACHE[S_FULL] = build_nc(S_FULL)
    nc = _NC_CACHE[S_FULL]

    in_maps = make_in_maps(query, key, value, Wq, bq, Wk, bk, Wv, bv, Wo, bo)
    res = None
    for attempt in range(3):
        try:
            res = bass_utils.run_bass_kernel_spmd(
                nc, in_maps, core_ids=list(range(N_CORES))
            )
            break
        except Exception:
            if attempt == 2:
                raise

    bo = np.asarray(bo, dtype=np.float32)
    y = np.zeros((B, S_FULL, D_MODEL), np.float32)
    for core in range(N_CORES):
        y[core // 4] += np.asarray(res.results[core]["y_out"], dtype=np.float32)
    y += bo[None, None, :]
    return y


# revision 18
# speedup vs baseline: 1.2379x; 1.2379x over previous
# Multi-head attention (B=2, S=4096, D=768, H=12) on 8 Trainium2 NeuronCores.
#
# Sharding: 24 (batch, head) units -> 3 heads x 1 batch per core.
#   core c: batch b = c // 4, heads h0..h0+2 where h0 = 3 * (c % 4).
# Each core computes q/k/v projections for its heads, attention, and a
# row-parallel partial of the output projection (its 192 columns of the
# concat dimension).  Host sums the 4 partials per batch and adds bo.
#
# Engine split for softmax exp (the co-bottleneck with the PE):
#   - lane-0 units exp on ScalarE (ACT), lane-1 units exp on VectorE via a
#     custom 8-op DVE instruction that constructs fp16 BITS of exp directly
#     (floor-capture magic-add + endpoint-constrained quadratic mantissa),
#     followed by an int16 constant-subtract; PV reads the int16 tile
#     bitcast as fp16.  A 256-column ACT helper slice balances engines.
#   - scores arrive pre-scaled by ALPHA (folded into Wk host-side); both
#     exp paths emit weights scaled by 2^-6 (cancels in softmax).
#   - softmax rowsums ride a ones-column in V (free on PE); reciprocals are
#     batched into one [rows,512] tile + reciprocal_approx_fast.
import numpy as np

D_MODEL = 768
NUM_HEADS = 12
DK = 64
B = 2
S_FULL = 4096
N_CORES = 8
HPC = 3  # heads per core
CT = D_MODEL // 128  # contraction tiles for projections

# ---- custom DVE exp-bits constants ----
M_MAGIC = 1.5 * 2**33
PRESHIFT = 17920.0  # Be(18432) - 512
C2_FIT = 0.00033312047729328157
K2_SUB = float(np.rint(8704.0 + 262144.0 * C2_FIT))
ALPHA = 1024.0 * 0.125 / np.log(2)  # raw-score -> bits scale (baked into Wk)
LN2 = float(np.log(2))
ACT_HELPER = 256
DEBUG_DUMP = False
FORCE_ACT = False  # columns of each DVE-stream chunk handled by ACT instead


def _exp_ref(in0, in1, s0, s1, imm2):
    y = (in0.astype(np.float32) + np.float32(s0)).astype(np.float32)
    t = (y + np.float32(s1)).astype(np.float32)
    r0 = (t - np.float32(s1)).astype(np.float32)
    h = (y - r0).astype(np.float32)
    q = (np.float32(imm2) * h).astype(np.float32)
    p = (q * h).astype(np.float32)
    u = (p + h).astype(np.float32)
    return (u + r0).astype(np.float32)


def make_exp_op():
    import concourse.dve_ops as dve_ops_mod
    from concourse.dve_ops import DveOp
    from concourse.dve_spec import C0, C1, C2, Spec, Src0, lower
    from concourse.dve_uop import DveOpSpec

    name = "EXP_BITS_ANT"
    if name in dve_ops_mod._SUB_OPCODE_FOR_NAME:
        return next(o for o in dve_ops_mod.OPS if o.name == name)
    y = Src0 + C0
    t = y + C1
    r0 = t - C1
    h = y - r0
    q = C2 * h
    p = q * h
    u = p + h
    body = u + r0
    spec = Spec(body=body, reference=_exp_ref)
    row = max(dve_ops_mod._SUB_OPCODE_FOR_NAME.values()) + 1
    assert row < 0x20, row
    tmp = DveOpSpec(name=name, opcode=row, uops=lower(spec, ver="v3"),
                    rd1_en=False)
    op = DveOp(name, spec, subdim=False, uops_sha={"v3": tmp.sha("v3")})
    dve_ops_mod.OPS.append(op)
    dve_ops_mod._SUB_OPCODE_FOR_NAME[name] = row
    dve_ops_mod.CUSTOM_DVE_SPECS[name] = op.spec
    return op


def _chunk_sizes(ktiles):
    # 3 k-tiles per exp chunk; two independent streams each own a 3-bank
    # psum slot + a 1-bank output accumulator (3+3+1+1 = 8 banks)
    out = []
    rem = ktiles
    if rem % 3:
        out.append(rem % 3)
        rem -= rem % 3
    while rem > 0:
        out.append(3)
        rem -= 3
    return out


def _emit(nc, tc, S):
    import concourse.mybir as mybir
    from contextlib import ExitStack

    exp_op = make_exp_op()
    f32 = mybir.dt.float32
    fr = mybir.dt.float16
    i16 = mybir.dt.int16
    u16 = mybir.dt.uint16
    Exp = mybir.ActivationFunctionType.Exp
    ADD = mybir.AluOpType.add
    SUB = mybir.AluOpType.subtract
    MAX = mybir.AluOpType.max

    QB = S // 512  # 512-query blocks
    ST = S // 128  # 128-row tiles of S (also k-tiles)
    CHUNKS = _chunk_sizes(ST)
    N_UNITS = HPC * QB

    # ---- DRAM I/O ----
    xq = nc.dram_tensor("xq_t", [D_MODEL, S], fr, kind="ExternalInput")
    xk = nc.dram_tensor("xk_t", [D_MODEL, S], fr, kind="ExternalInput")
    xv = nc.dram_tensor("xv_t", [D_MODEL, S], fr, kind="ExternalInput")
    wq = nc.dram_tensor("wq_t", [D_MODEL, 256], fr, kind="ExternalInput")
    wk = nc.dram_tensor("wk_t", [D_MODEL, 256], fr, kind="ExternalInput")
    wv = nc.dram_tensor("wv_t", [D_MODEL, 256], fr, kind="ExternalInput")
    wo = nc.dram_tensor("wo_t", [DK, HPC, D_MODEL], fr, kind="ExternalInput")
    bqd = nc.dram_tensor("bq_p", [128, 2], f32, kind="ExternalInput")
    bkd = nc.dram_tensor("bk_p", [128, 2], f32, kind="ExternalInput")
    bvd = nc.dram_tensor("bv_p", [128, HPC * DK], f32, kind="ExternalInput")
    y_out = nc.dram_tensor("y_out", [S, D_MODEL], fr, kind="ExternalOutput")

    ctx = ExitStack()
    with ctx:
        persist = ctx.enter_context(tc.tile_pool(name="persist", bufs=1))
        xpool = ctx.enter_context(tc.tile_pool(name="xpool", bufs=4))
        ptpool = ctx.enter_context(tc.tile_pool(name="ptpool", bufs=4))
        spool = ctx.enter_context(tc.tile_pool(name="spool", bufs=2))
        ps = ctx.enter_context(tc.tile_pool(name="ps", bufs=1, space="PSUM"))

        def s_slot(i):
            return ps.tile([128, 1536], f32, tag=("s3a" if i % 2 == 0 else "s3b"),
                           name=f"sslot{i % 2}")

        def o_slot(i):
            return ps.tile([128, 512], f32, tag=("oa" if i % 2 == 0 else "ob"),
                           name=f"oslot{i % 2}")

        # ---- persistent SBUF ----
        wq_sb = persist.tile([128, CT, 256], fr, tag="wq_sb")
        wk_sb = persist.tile([128, CT, 256], fr, tag="wk_sb")
        wv_sb = persist.tile([128, CT, 256], fr, tag="wv_sb")
        wo_sb = persist.tile([DK, HPC, D_MODEL], fr, tag="wo_sb")
        bq_sb = persist.tile([128, 2], f32, tag="bq_sb")
        bk_sb = persist.tile([128, 2], f32, tag="bk_sb")
        bv_sb = persist.tile([128, HPC * DK], f32, tag="bv_sb")
        ones_sb = persist.tile([128, DK], fr, tag="ones_sb")
        qt01 = persist.tile([128, S], fr, tag="qt01")
        qt2 = persist.tile([128, S], fr, tag="qt2")
        kt01 = persist.tile([128, S], fr, tag="kt01")
        kt2 = persist.tile([128, S], fr, tag="kt2")
        v_all = persist.tile([128, ST, HPC, DK + 1], fr, tag="v_all")
        ot = [
            persist.tile([DK + 1, S], fr, tag=f"ot{h}", name=f"ot{h}")
            for h in range(HPC)
        ]
        ebias = persist.tile([128, 1], f32, tag="ebias")
        rs_all = persist.tile([N_UNITS, 512], fr, tag="rs_all")
        rs32 = persist.tile([N_UNITS, 512], f32, tag="rs32")
        rr_all = persist.tile([N_UNITS, 512], f32, tag="rr_all")
        rr16 = persist.tile([N_UNITS, 512], fr, tag="rr16")

        nc.sync.dma_start(wq_sb[:], wq[:].rearrange("(o p) m -> p o m", p=128))
        nc.sync.dma_start(wk_sb[:], wk[:].rearrange("(o p) m -> p o m", p=128))
        nc.sync.dma_start(wv_sb[:], wv[:].rearrange("(o p) m -> p o m", p=128))
        nc.sync.dma_start(wo_sb[:], wo[:])
        nc.sync.dma_start(bq_sb[:], bqd[:])
        nc.sync.dma_start(bk_sb[:], bkd[:])
        nc.sync.dma_start(bv_sb[:], bvd[:])
        nc.vector.memset(ones_sb[:], 1.0)
        nc.vector.memset(ebias[:], -6.0 * LN2)
        nc.vector.memset(v_all[:, :, :, DK : DK + 1], 1.0)

        # ---- q/k projections (transposed form [heads*64, S]) ----
        def proj_qk_block(x_dram, w_sb, b_sb, dst01, dst2, qb, xtag):
            sl = slice(qb * 512, (qb + 1) * 512)
            xt = xpool.tile([128, CT, 512], fr, tag=xtag, name=f"xt_{xtag}")
            nc.sync.dma_start(
                xt[:], x_dram[:, sl].rearrange("(o p) s -> p o s", p=128)
            )
            slot = s_slot(qb)
            p1 = slot[:, 0:512]
            p2 = slot[:, 512:1024]
            for c in range(CT):
                nc.tensor.matmul(
                    p1, w_sb[:, c, 0:128], xt[:, c, :],
                    start=(c == 0), stop=(c == CT - 1),
                )
                nc.tensor.matmul(
                    p2, w_sb[:, c, 128:256], xt[:, c, :],
                    start=(c == 0), stop=(c == CT - 1),
                )
            nc.vector.tensor_scalar(dst01[:, sl], p1, b_sb[:, 0:1], None, ADD)
            nc.vector.tensor_scalar(dst2[:, sl], p2, b_sb[:, 1:2], None, ADD)

        # order: k first, then v, then q
        for qb in range(QB):
            proj_qk_block(xk, wk_sb, bk_sb, kt01, kt2, qb, "xk")

        # ---- v projection (natural layout [S, 64] per head) ----
        for g in range(ST // 4):
            gsl = slice(g * 512, (g + 1) * 512)
            xt = xpool.tile([128, CT, 512], fr, tag="xv")
            nc.sync.dma_start(
                xt[:], xv[:, gsl].rearrange("(o p) s -> p o s", p=128)
            )
            for st in range(g * 4, g * 4 + 4):
                off = (st % 4) * 128
                pv = s_slot(st)[:, 0:256]
                for c in range(CT):
                    nc.tensor.matmul(
                        pv, xt[:, c, off : off + 128], wv_sb[:, c, 0:256],
                        start=(c == 0), stop=(c == CT - 1),
                    )
                for h in range(HPC):
                    nc.vector.tensor_add(
                        v_all[:, st, h, 0:DK],
                        pv[:, h * DK : (h + 1) * DK],
                        bv_sb[:, h * DK : (h + 1) * DK],
                    )

        for qb in range(QB):
            proj_qk_block(xq, wq_sb, bq_sb, qt01, qt2, qb, "xq")

        # ---- attention: paired streams, QK packed as concurrent row-groups ----
        # pair (h0,qb)+(h1,qb): h0 on array rows 0-63, h1 on rows 64-127
        # pair (h2,qb)+(h2,qb'): uses qt2/kt2 whose rows 64-127 duplicate h2
        # lane 0 exp on ACT; lane 1 exp on DVE custom op (+ACT helper cols)
        def unit_aps(h, lane):
            rows = slice(0, DK) if lane == 0 else slice(DK, 128)
            if h < 2:
                return (qt01[rows, :], kt01[rows, :])
            return (qt2[rows, :], kt2[rows, :])

        def unit_state(h, qb, idx, lane):
            qt_ap, kt_ap = unit_aps(h, lane)
            return {
                "h": h, "sl": slice(qb * 512, (qb + 1) * 512),
                "po": o_slot(idx), "kk": 0, "qt": qt_ap, "kt": kt_ap,
                "lane": lane,
            }

        def emit_chunk_qk(p_s, st_, j):
            kk = st_["kk"]
            kt_sl = slice((kk + j) * 128, (kk + j + 1) * 128)
            nc.tensor.matmul(
                p_s[:, j * 512 : (j + 1) * 512],
                st_["kt"][:, kt_sl], st_["qt"][:, st_["sl"]],
                start=True, stop=True,
            )

        def emit_chunk_act(p_s, idx, st_, cs):
            W = cs * 512
            if st_["lane"] == 0 or FORCE_ACT:
                pt = ptpool.tile([128, 1536], fr, tag=f"ptA{idx}",
                                 name=f"ptA{idx}")
                nc.scalar.activation(pt[:, :W], p_s[:, :W], Exp,
                                     bias=ebias[:], scale=LN2 / 1024.0)
                st_.setdefault("pv_q", []).append((pt, None, st_["kk"], cs))
            else:
                raw = ptpool.tile([128, 1536], u16, tag="rawB", name="rawB")
                pt = ptpool.tile([128, 1536], i16, tag=f"ptB{idx % 2}",
                                 name=f"ptB{idx % 2}")
                split = W - ACT_HELPER
                nc.vector._custom_dve(
                    exp_op, out=raw[:, 0:split], in0=p_s[:, 0:split],
                    s0=PRESHIFT, s1=M_MAGIC, imm2=C2_FIT,
                )
                nc.vector.tensor_scalar(
                    pt[:, 0:split], raw[:, 0:split], K2_SUB, 0.0, SUB, MAX
                )
                if ACT_HELPER:
                    nc.scalar.activation(
                        pt[:, split:W].bitcast(fr), p_s[:, split:W], Exp,
                        bias=ebias[:], scale=LN2 / 1024.0,
                    )
                st_.setdefault("pv_q", []).append((pt, fr, st_["kk"], cs))
            st_["kk"] += cs

        def emit_pv(st_, flush=False, depth=2):
            q = st_.setdefault("pv_q", [])
            while q and (flush or len(q) > depth - 1):
                pt, cast, kk, cs = q.pop(0)
                h, po = st_["h"], st_["po"]
                pt_ap = pt[:].bitcast(cast) if cast is not None else pt[:]
                for j in range(cs):
                    nc.tensor.matmul(
                        po[0 : DK + 1, :],
                        v_all[:, kk + j, h, :],
                        pt_ap[:, j * 512 : (j + 1) * 512],
                        start=(kk + j == 0), stop=(kk + j == ST - 1),
                    )

        # finish: copy PV psum out, stash rowsum row; recip + normalize are
        # batched per 2 pairs (4 units)
        norm_pend = []
        rows_done = [0]  # units copied so far (== rs_all rows filled)

        def finish_copy(st_, row):
            h, sl, po = st_["h"], st_["sl"], st_["po"]
            nc.vector.tensor_copy(ot[h][0 : DK + 1, sl], po[0 : DK + 1, :])
            nc.sync.dma_start(rs_all[row : row + 1, :], ot[h][DK : DK + 1, sl])
            norm_pend.append((st_["h"], st_["sl"], row))
            rows_done[0] = row + 1

        def flush_norms():
            if not norm_pend:
                return
            # whole-tile ops: DVE cost is per-lane FD, and engine APs must be
            # 32-partition aligned; unfilled rows are garbage-in/garbage-out
            nc.vector.tensor_copy(rs32[:], rs_all[:])
            nc.vector.reciprocal_approx_fast(rr_all[:], rs32[:])
            nc.vector.tensor_copy(rr16[:], rr_all[:])
            for h, sl, row in norm_pend:
                rb1 = spool.tile([1, 512], fr, tag="rb1")
                nc.sync.dma_start(rb1[:], rr16[row : row + 1, :])
                rbc = spool.tile([DK, 512], fr, tag="rbc")
                nc.gpsimd.partition_broadcast(rbc[:], rb1[0:1, :])
                nc.vector.tensor_mul(ot[h][0:DK, sl], ot[h][0:DK, sl], rbc[:])
            norm_pend.clear()

        pairs = [((0, qb), (1, qb)) for qb in range(QB)]
        h2qbs = list(range(QB))
        while len(h2qbs) >= 2:
            pairs.append(((2, h2qbs.pop(0)), (2, h2qbs.pop(0))))
        solo = [(2, qb) for qb in h2qbs]

        def emit_y(qts):
            for qt in qts:
                q_sl = slice(qt * 128, (qt + 1) * 128)
                py = s_slot(qt)[:, 0:768]
                for h in range(HPC):
                    nc.tensor.matmul(
                        py[:, 0:512], ot[h][0:DK, q_sl], wo_sb[:, h, 0:512],
                        start=(h == 0), stop=(h == HPC - 1),
                    )
                    nc.tensor.matmul(
                        py[:, 512:768], ot[h][0:DK, q_sl], wo_sb[:, h, 512:768],
                        start=(h == 0), stop=(h == HPC - 1),
                    )
                ysb = spool.tile([128, D_MODEL], fr, tag="ysb", bufs=3)
                nc.scalar.copy(ysb[:], py)
                nc.sync.dma_start(y_out[q_sl, :], ysb[:])

        pending = None
        for pi, ((hA, qbA), (hB, qbB)) in enumerate(pairs):
            stA = unit_state(hA, qbA, 0, 0)
            stB = unit_state(hB, qbB, 1, 1)
            for ci, cs in enumerate(CHUNKS):
                psA = s_slot(0)
                psB = s_slot(1)
                for j in range(cs):
                    emit_chunk_qk(psA, stA, j)
                    emit_chunk_qk(psB, stB, j)
                emit_pv(stA)
                emit_pv(stB)
                if DEBUG_DUMP and pi == 0 and ci == 1:
                    ps_d = nc.dram_tensor("psB_dump", [128, 1536], f32,
                                          kind="ExternalOutput")
                    ps_stage = persist.tile([128, 1536], f32, tag="ps_stage")
                    nc.vector.tensor_copy(ps_stage[:], psB[:])
                    nc.sync.dma_start(ps_d[:], ps_stage[:])
                emit_chunk_act(psA, 0, stA, cs)
                emit_chunk_act(psB, 1, stB, cs)
                if DEBUG_DUMP and pi == 0 and ci == 1:
                    ptB_dbg = stB["pv_pend"][0]
                    pt_d = nc.dram_tensor("ptB_dump", [128, 1536], i16,
                                          kind="ExternalOutput")
                    nc.sync.dma_start(pt_d[:], ptB_dbg[:])
                if ci == 0 and pending is not None:
                    r = rows_done[0]
                    finish_copy(pending[0], r)
                    finish_copy(pending[1], r + 1)
                    pending = None
                    if len(norm_pend) >= 8:
                        flush_norms()
            emit_pv(stA, flush=True)
            emit_pv(stB, flush=True)
            pending = (stA, stB)
        if pending is not None:
            r = rows_done[0]
            finish_copy(pending[0], r)
            finish_copy(pending[1], r + 1)
            pending = None
        for h, qb in solo:
            stA = unit_state(h, qb, 0, 0)
            for ci, cs in enumerate(CHUNKS):
                psA = s_slot(0)
                for j in range(cs):
                    emit_chunk_qk(psA, stA, j)
                emit_pv(stA)
                emit_chunk_act(psA, 0, stA, cs)
            emit_pv(stA, flush=True)
            finish_copy(stA, rows_done[0])
        flush_norms()

        # ---- output projection partials ----
        emit_y(range(ST))

        if DEBUG_DUMP:
            rs_d = nc.dram_tensor("rs_dump", [N_UNITS, 512], fr,
                                  kind="ExternalOutput")
            rr_d = nc.dram_tensor("rr_dump", [N_UNITS, 512], f32,
                                  kind="ExternalOutput")
            nc.sync.dma_start(rs_d[:], rs_all[:])
            nc.sync.dma_start(rr_d[:], rr_all[:])
            for h in range(HPC):
                ot_d = nc.dram_tensor(f"ot_dump{h}", [DK + 1, S], fr,
                                      kind="ExternalOutput")
                nc.sync.dma_start(ot_d[:], ot[h][:])


def build_nc(S=S_FULL):
    import concourse.bacc as bacc
    import concourse.tile as tile

    nc = bacc.Bacc("TRN2", target_bir_lowering=False, debug=False)
    with tile.TileContext(nc) as tc:
        _emit(nc, tc, S)
    nc.compile()
    return nc


def make_in_maps(query, key, value, Wq, bq, Wk, bk, Wv, bv, Wo, bo, S=S_FULL):
    """Per-core input dicts (host-side sharding / layout marshalling)."""
    query = np.asarray(query, dtype=np.float32)
    key = np.asarray(key, dtype=np.float32)
    value = np.asarray(value, dtype=np.float32)
    Wq, Wk, Wv, Wo = (np.asarray(w, dtype=np.float32) for w in (Wq, Wk, Wv, Wo))
    bq, bk, bv = (np.asarray(x, dtype=np.float32) for x in (bq, bk, bv))

    xq_b = [np.ascontiguousarray(query[b].T.astype(np.float16)) for b in range(B)]
    xk_b = [np.ascontiguousarray(key[b].T.astype(np.float16)) for b in range(B)]
    xv_b = [np.ascontiguousarray(value[b].T.astype(np.float16)) for b in range(B)]
    # K-projection carries the ALPHA score pre-scale for the exp bit trick
    WqT = Wq.T.astype(np.float16)
    WkT = (Wk.T * np.float32(ALPHA)).astype(np.float16)
    WvT = Wv.T.astype(np.float16)
    WoT = Wo.T.astype(np.float16)
    bk_s = bk * np.float32(ALPHA)

    in_maps = []
    for core in range(N_CORES):
        b = core // 4
        h0 = HPC * (core % 4)
        cs = slice(h0 * DK, (h0 + HPC) * DK)
        bq_p = np.zeros((128, 2), np.float32)
        bk_p = np.zeros((128, 2), np.float32)
        bq_l, bk_l, bv_l = bq[cs], bk_s[cs], bv[cs]
        bq_p[:, 0], bq_p[0:DK, 1], bq_p[DK:128, 1] = (
            bq_l[0:128], bq_l[128:192], bq_l[128:192])
        bk_p[:, 0], bk_p[0:DK, 1], bk_p[DK:128, 1] = (
            bk_l[0:128], bk_l[128:192], bk_l[128:192])
        in_maps.append({
            "xq_t": xq_b[b],
            "xk_t": xk_b[b],
            "xv_t": xv_b[b],
            "wq_t": np.concatenate(
                [WqT[:, cs], WqT[:, cs.start + 2 * DK : cs.stop]], axis=1
            ),
            "wk_t": np.concatenate(
                [WkT[:, cs], WkT[:, cs.start + 2 * DK : cs.stop]], axis=1
            ),
            "wv_t": np.concatenate(
                [WvT[:, cs], np.zeros((D_MODEL, 256 - HPC * DK), np.float16)], axis=1
            ),
            "wo_t": np.ascontiguousarray(
                WoT[cs, :].reshape(HPC, DK, D_MODEL).transpose(1, 0, 2)
            ),
            "bq_p": bq_p,
            "bk_p": bk_p,
            "bv_p": np.tile(bv_l[None, :], (128, 1)).astype(np.float32),
        })
    return in_maps


_NC_CACHE = {}


def kernel(query, key, value, Wq, bq, Wk, bk, Wv, bv, Wo, bo):
    from concourse import bass_utils

    if S_FULL not in _NC_CACHE:
        _NC_CACHE[S_FULL] = build_nc(S_FULL)
    nc = _NC_CACHE[S_FULL]

    in_maps = make_in_maps(query, key, value, Wq, bq, Wk, bk, Wv, bv, Wo, bo)
    res = None
    for attempt in range(3):
        try:
            res = bass_utils.run_bass_kernel_spmd(
                nc, in_maps, core_ids=list(range(N_CORES))
            )
            break
        except Exception:
            if attempt == 2:
                raise

    bo = np.asarray(bo, dtype=np.float32)
    y = np.zeros((B, S_FULL, D_MODEL), np.float32)
    for core in range(N_CORES):
        y[core // 4] += np.asarray(res.results[core]["y_out"], dtype=np.float32)
    y += bo[None, None, :]
    return y
